# revision 5
# baseline (speedup 1.0000x reference)
"""Trainium2 Bass kernel for nn_CompositeEmbeddingA (octree composite embedding).

Per sample (1 sample per NeuronCore, batch=8 over 8 cores):
  layers 0-2 (depths 1-3): x = val_emb[v] + pos0[p0] + pos1[p1] + pos2[p2] + dep_emb[d]
  layers 3-4: same sum w/o dep, then Conv1d(E,E,kernel=stride=k), k=4 (l3) / 8 (l4)

Algorithm: every layer is expressed as  out = MultiHot @ Table  on the PE:
  - conv folded into the tables host-side: per tap j, T_j = concat(tables) @ w[:,:,j].T,
    so out[t] = sum_j multihot(token 8t+j) @ T_j  == one K=(196k) matmul per layer.
  - MultiHot^T (contraction dim on partitions) is built on-chip:
      PE "broadcast matmul": bcast[r_row, tok] = selector^T @ idx_rows  (replicates the
      right index value into every table row), then DVE is_equal against a per-partition
      constant column -> exact 0/1 one-hot, fp32.
  - conv bias = one extra table row whose selector column is all-zero (bcast value 0)
    with compare const 0 -> fires for every token.
  - main matmuls run in float32r (full fp32 data, 1 cycle/row at N>=256).
"""

import sys

for _p in ("/opt/trn_rl_repo",):
    if _p not in sys.path:
        sys.path.insert(0, _p)

import numpy as np
import ml_dtypes

RES = 32
SPATIAL = 3
NUM_VOCAB = 3
E = 256
BATCH = 8
LAYER_SIZES = (8, 64, 512, 4096, 32768)
CONV_SIZE = {3: 4, 4: 8}
S_TOTAL = sum(LAYER_SIZES)  # 37448
OUT_TOKENS = 8 + 64 + 512 + 1024 + 4096  # 5704
NIDX = 32  # uniform idx-row count for the broadcast matmul
STRIPE = 512

# segment widths inside one tap: value(4), pos0(64), pos1(64), pos2(64) [, dep(6)]
SEG_W = (NUM_VOCAB + 1, 2 * RES, 2 * RES, 2 * RES)
DEP_W = 6

_BF16 = ml_dtypes.bfloat16


def _layer_slices():
    out = []
    start = 0
    for n in LAYER_SIZES:
        out.append((start, start + n))
        start += n
    return out


LAYER_SL = _layer_slices()


def _build_consts(params):
    """Fold conv weights into tables; pack rows into 128-row chunks.

    Returns (tbl [NC,128,256] f32, sel [NC,32,128] bf16, cval [NC,128,1] f32,
             layers: list of (name, T_tokens, out_offset, chunk_index_list))
    """
    rows_tbl = []   # per logical row: the 256-vector
    rows_ridx = []  # which of the 32 idx rows feeds this row (-1 = none: bcast val 0)
    rows_c = []     # compare constant
    layer_marks = []  # (row_start, row_end) per virtual layer

    def seg_tables(l):
        t = [np.asarray(params[f"val_emb_{l}"], np.float32)]
        pe = np.asarray(params[f"pos_emb_{l}"], np.float32)
        t += [pe[0], pe[1], pe[2]]
        return t

    # virtual layer "B": real layers 0..2 merged. idx rows: l*5 + (v,p0,p1,p2,d)
    r0 = len(rows_tbl)
    for l in range(3):
        tabs = seg_tables(l) + [np.asarray(params[f"dep_emb_{l}"], np.float32)]
        for seg, tab in enumerate(tabs):
            for c in range(tab.shape[0]):
                rows_tbl.append(tab[c])
                rows_ridx.append(l * 5 + seg)
                rows_c.append(float(c))
    layer_marks.append((r0, len(rows_tbl)))

    # conv layers: idx rows j*4+seg; one bias row (all-zero selector col, c=0)
    for l in (3, 4):
        r0 = len(rows_tbl)
        k = CONV_SIZE[l]
        w = np.asarray(params[f"conv_w_{l}"], np.float32)  # [O, E, k]
        b = np.asarray(params[f"conv_b_{l}"], np.float32)  # [O]
        tabs = seg_tables(l)
        for j in range(k):
            wj = w[:, :, j]  # [O, E]
            for seg, tab in enumerate(tabs):
                folded = tab @ wj.T  # [rows, O]
                for c in range(tab.shape[0]):
                    rows_tbl.append(folded[c])
                    rows_ridx.append(j * 4 + seg)
                    rows_c.append(float(c))
        rows_tbl.append(b)
        rows_ridx.append(-1)
        rows_c.append(0.0)
        layer_marks.append((r0, len(rows_tbl)))

    # chunkify each virtual layer into 128-row chunks
    tbl_chunks, sel_chunks, cval_chunks = [], [], []
    layers = []
    out_offs = [0, 584, 1608]
    names = ["B", "L3", "L4"]
    t_counts = [584, 1024, 4096]
    for vl, (r0, r1) in enumerate(layer_marks):
        n = r1 - r0
        nch = -(-n // 128)
        cids = []
        for ci in range(nch):
            a = r0 + ci * 128
            bnd = min(r0 + (ci + 1) * 128, r1)
            rows = bnd - a
            tbl = np.zeros((128, E), np.float32)
            sel = np.zeros((NIDX, 128), np.float32)
            cv = np.full((128, 1), -1.0, np.float32)
            for m in range(rows):
                tbl[m] = rows_tbl[a + m]
                if rows_ridx[a + m] >= 0:
                    sel[rows_ridx[a + m], m] = 1.0
                cv[m, 0] = rows_c[a + m]
            cids.append(len(tbl_chunks))
            tbl_chunks.append(tbl)
            sel_chunks.append(sel.astype(_BF16))
            cval_chunks.append(cv)
        layers.append((names[vl], t_counts[vl], out_offs[vl], cids))

    return (
        np.stack(tbl_chunks),
        np.stack(sel_chunks),
        np.stack(cval_chunks),
        layers,
    )


def _build_ridx(value, depth, position, b):
    """Per-core index-row tensors, one per virtual layer: [32, T] bf16."""
    out = {}
    # B: merged layers 0-2; out tokens 0..583 = input tokens 0..583
    rb = np.full((NIDX, 584), -1.0, np.float32)
    col = 0
    for l in range(3):
        lo, hi = LAYER_SL[l]
        n = hi - lo
        rb[l * 5 + 0, col : col + n] = value[b, lo:hi]
        for s in range(3):
            rb[l * 5 + 1 + s, col : col + n] = position[b, lo:hi, s]
        rb[l * 5 + 4, col : col + n] = depth[b, lo:hi]
        col += n
    out["B"] = rb.astype(_BF16)
    for name, l in (("L3", 3), ("L4", 4)):
        k = CONV_SIZE[l]
        lo, hi = LAYER_SL[l]
        T = (hi - lo) // k
        r = np.zeros((NIDX, T), np.float32)
        for j in range(k):
            r[j * 4 + 0] = value[b, lo:hi][j::k]
            for s in range(3):
                r[j * 4 + 1 + s] = position[b, lo:hi, s][j::k]
        out[name] = r.astype(_BF16)
    return out


_CACHE = {}


def _get_nc(layers, nchunks):
    key = ("v1", tuple((n, t, o, tuple(c)) for n, t, o, c in layers))
    if key in _CACHE:
        return _CACHE[key]

    import concourse.bass as bass
    import concourse.tile as tile
    from concourse import bacc, mybir
    from contextlib import ExitStack

    f32 = mybir.dt.float32
    f32r = mybir.dt.float32r
    bf16 = mybir.dt.bfloat16

    nc = bacc.Bacc(trn_type="TRN2", target_bir_lowering=False, debug=False)
    tbl_d = nc.dram_tensor("tbl", [nchunks, 128, E], f32r, kind="ExternalInput").ap()
    sel_d = nc.dram_tensor("sel", [nchunks, NIDX, 128], bf16, kind="ExternalInput").ap()
    cval_d = nc.dram_tensor("cval", [nchunks, 128, 1], f32, kind="ExternalInput").ap()
    ridx_d = {
        name: nc.dram_tensor(f"ridx_{name}", [NIDX, T], bf16, kind="ExternalInput").ap()
        for name, T, _, _ in layers
    }
    out_d = nc.dram_tensor("out", [OUT_TOKENS, E], f32, kind="ExternalOutput").ap()

    with tile.TileContext(nc) as tc, ExitStack() as ctx:
        cpool = ctx.enter_context(tc.tile_pool(name="const", bufs=1))
        rpool = ctx.enter_context(tc.tile_pool(name="ridx", bufs=3))
        mpool = ctx.enter_context(tc.tile_pool(name="mh", bufs=2))
        bps = ctx.enter_context(
            tc.tile_pool(name="bps", bufs=4, space=bass.MemorySpace.PSUM)
        )
        ops = ctx.enter_context(
            tc.tile_pool(name="ops", bufs=4, space=bass.MemorySpace.PSUM)
        )
        opool = ctx.enter_context(tc.tile_pool(name="osb", bufs=6))

        tbl_t, sel_t, cv_t = [], [], []
        for ci in range(nchunks):
            t = cpool.tile([128, E], f32r, tag=f"tbl{ci}")
            nc.sync.dma_start(t[:], tbl_d[ci])
            tbl_t.append(t)
            s = cpool.tile([NIDX, 128], bf16, tag=f"sel{ci}")
            nc.sync.dma_start(s[:], sel_d[ci])
            sel_t.append(s)
            c = cpool.tile([128, 1], f32, tag=f"cv{ci}")
            nc.sync.dma_start(c[:], cval_d[ci])
            cv_t.append(c)

        for name, T, out_off, cids in layers:
            for s0 in range(0, T, STRIPE):
                W = min(STRIPE, T - s0)
                rt = rpool.tile([NIDX, W], bf16, tag="r")
                nc.sync.dma_start(rt[:], ridx_d[name][:, s0 : s0 + W])
                mhs = []
                for k, ci in enumerate(cids):
                    bp = bps.tile([128, W], f32, tag="b")
                    nc.tensor.matmul(bp[:], sel_t[ci][:], rt[:], start=True, stop=True)
                    mh = mpool.tile([128, W], f32r, tag=f"mh{k}")
                    nc.vector.tensor_scalar(
                        mh[:], bp[:], cv_t[ci][:], None, op0=mybir.AluOpType.is_equal
                    )
                    mhs.append(mh)
                for t0 in range(0, W, 128):
                    M = min(128, W - t0)
                    op = ops.tile([M, E], f32, tag="o")
                    for k, ci in enumerate(cids):
                        nc.tensor.matmul(
                            op[:],
                            mhs[k][:, t0 : t0 + M],
                            tbl_t[ci][:],
                            start=(k == 0),
                            stop=(k == len(cids) - 1),
                        )
                    ob = opool.tile([M, E], f32, tag="ob")
                    nc.scalar.activation(
                        ob[:], op[:], mybir.ActivationFunctionType.Copy
                    )
                    row = out_off + s0 + t0
                    nc.sync.dma_start(out_d[row : row + M, :], ob[:])

    nc.compile()
    _CACHE[key] = nc
    return nc


def kernel(**inputs):
    from concourse.bass_utils import run_bass_kernel_spmd

    value = np.asarray(inputs["value"], np.int32).astype(np.float32)
    depth = np.asarray(inputs["depth"], np.int32).astype(np.float32)
    position = np.asarray(inputs["position"], np.int32).astype(np.float32)

    tbl, sel, cval, layers = _build_consts(inputs)
    nc = _get_nc(layers, tbl.shape[0])

    in_maps = []
    for b in range(BATCH):
        rid = _build_ridx(value, depth, position, b)
        m = {"tbl": tbl, "sel": sel, "cval": cval}
        for name, _, _, _ in layers:
            m[f"ridx_{name}"] = rid[name]
        in_maps.append(m)

    res = run_bass_kernel_spmd(nc, in_maps, list(range(BATCH)))
    return np.stack([res.results[b]["out"] for b in range(BATCH)])


# revision 36
# speedup vs baseline: 1.4268x; 1.4268x over previous
"""Trainium2 Bass kernel for nn_CompositeEmbeddingA (octree composite embedding).

Per sample (1 sample per NeuronCore, batch=8 over 8 cores):
  layers 0-2 (depths 1-3): x = val_emb[v] + pos0[p0] + pos1[p1] + pos2[p2] + dep_emb[d]
  layers 3-4: same sum w/o dep, then Conv1d(E,E,kernel=stride=k), k=4 (l3) / 8 (l4)

Algorithm: every layer is expressed as  out = MultiHot @ Table  on the PE:
  - conv folded into the tables host-side: per tap j, T_j = concat(tables) @ w[:,:,j].T,
    so out[t] = sum_j multihot(token 8t+j) @ T_j  == one K=(196k) matmul per layer.
  - MultiHot^T (contraction dim on partitions) is built on-chip:
      PE "broadcast matmul": bcast[r_row, tok] = selector^T @ idx_rows  (replicates the
      right index value into every table row), then DVE is_equal against a per-partition
      constant column -> exact 0/1 one-hot, fp32.
  - conv bias = one extra table row whose selector column is all-zero (bcast value 0)
    with compare const 0 -> fires for every token.
  - main matmuls run in float32r (full fp32 data, 1 cycle/row at N>=256).
"""

import sys

for _p in ("/opt/trn_rl_repo",):
    if _p not in sys.path:
        sys.path.insert(0, _p)

import numpy as np
import ml_dtypes

RES = 32
SPATIAL = 3
NUM_VOCAB = 3
E = 256
BATCH = 8
LAYER_SIZES = (8, 64, 512, 4096, 32768)
CONV_SIZE = {3: 4, 4: 8}
S_TOTAL = sum(LAYER_SIZES)  # 37448
OUT_TOKENS = 8 + 64 + 512 + 1024 + 4096  # 5704
NIDX = 33  # 32 idx rows + one all-ones row (carries the -c compare constants)
ONES_ROW = 32
STRIPE = 512

# segment widths inside one tap: value(4), pos0(64), pos1(64), pos2(64) [, dep(6)]
SEG_W = (NUM_VOCAB + 1, 2 * RES, 2 * RES, 2 * RES)
DEP_W = 6

_BF16 = ml_dtypes.bfloat16


def _layer_slices():
    out = []
    start = 0
    for n in LAYER_SIZES:
        out.append((start, start + n))
        start += n
    return out


LAYER_SL = _layer_slices()


def _build_consts(params):
    """Fold conv weights into tables; pack rows into 128-row chunks.

    Returns (tbl [NC,128,256] f32, sel [NC,32,128] bf16, cval [NC,128,1] f32,
             layers: list of (name, T_tokens, out_offset, chunk_index_list))
    """
    rows_tbl = []   # per logical row: the 256-vector
    rows_ridx = []  # which of the 32 idx rows feeds this row (-1 = none: bcast val 0)
    rows_c = []     # compare constant
    layer_marks = []  # (row_start, row_end) per virtual layer

    def seg_tables(l):
        t = [np.asarray(params[f"val_emb_{l}"], np.float32)]
        pe = np.asarray(params[f"pos_emb_{l}"], np.float32)
        t += [pe[0], pe[1], pe[2]]
        return t

    # virtual layer "B": real layers 0..2 merged. idx rows: l*5 + (v,p0,p1,p2,d)
    r0 = len(rows_tbl)
    for l in range(3):
        tabs = seg_tables(l) + [np.asarray(params[f"dep_emb_{l}"], np.float32)]
        for seg, tab in enumerate(tabs):
            for c in range(tab.shape[0]):
                rows_tbl.append(tab[c])
                rows_ridx.append(l * 5 + seg)
                rows_c.append(float(c))
    layer_marks.append((r0, len(rows_tbl)))

    # conv layers: idx rows j*4+seg; one bias row (all-zero selector col, c=0)
    for l in (3, 4):
        r0 = len(rows_tbl)
        k = CONV_SIZE[l]
        w = np.asarray(params[f"conv_w_{l}"], np.float32)  # [O, E, k]
        b = np.asarray(params[f"conv_b_{l}"], np.float32)  # [O]
        tabs = seg_tables(l)
        for j in range(k):
            wj = w[:, :, j]  # [O, E]
            for seg, tab in enumerate(tabs):
                folded = tab @ wj.T  # [rows, O]
                for c in range(tab.shape[0]):
                    rows_tbl.append(folded[c])
                    rows_ridx.append(j * 4 + seg)
                    rows_c.append(float(c))
        rows_tbl.append(b)
        rows_ridx.append(-1)
        rows_c.append(0.0)
        layer_marks.append((r0, len(rows_tbl)))

    # chunkify each virtual layer into 128-row chunks
    tbl_chunks, sel_chunks, cval_chunks = [], [], []
    layers = []
    out_offs = [0, 584, 1608]
    names = ["B", "L3", "L4"]
    t_counts = [584, 1024, 4096]
    for vl, (r0, r1) in enumerate(layer_marks):
        n = r1 - r0
        nch = -(-n // 128)
        cids = []
        for ci in range(nch):
            a = r0 + ci * 128
            bnd = min(r0 + (ci + 1) * 128, r1)
            rows = bnd - a
            tbl = np.zeros((128, E), np.float32)
            sel = np.zeros((NIDX, 128), np.float32)
            sel[ONES_ROW, :] = 1.0  # pad rows: bcast value = +1 -> eq(.,0)=0
            for m in range(rows):
                tbl[m] = rows_tbl[a + m]
                if rows_ridx[a + m] >= 0:
                    sel[rows_ridx[a + m], m] = 1.0
                # ones-row coefficient: broadcast out = idx - c
                sel[ONES_ROW, m] = -rows_c[a + m]
            cids.append(len(tbl_chunks))
            tbl_chunks.append(tbl)
            sel_chunks.append(sel.astype(_BF16))
        layers.append((names[vl], t_counts[vl], out_offs[vl], cids))

    # merged layouts: one DMA per constant tensor
    tbl = np.concatenate(tbl_chunks, axis=1)  # [128, NC*256] f32
    sel = np.concatenate(sel_chunks, axis=1)  # [33, NC*128] bf16
    return tbl, sel, layers


def _build_ridx(value, depth, position, b):
    """Per-core index-row tensors, one per virtual layer: [32, T] bf16."""
    out = {}
    # B: merged layers 0-2; out tokens 0..583 = input tokens 0..583
    rb = np.full((NIDX, 584), -1.0, np.float32)
    rb[ONES_ROW] = 1.0
    col = 0
    for l in range(3):
        lo, hi = LAYER_SL[l]
        n = hi - lo
        rb[l * 5 + 0, col : col + n] = value[b, lo:hi]
        for s in range(3):
            rb[l * 5 + 1 + s, col : col + n] = position[b, lo:hi, s]
        rb[l * 5 + 4, col : col + n] = depth[b, lo:hi]
        col += n
    out["B"] = rb.astype(_BF16)
    for name, l in (("L3", 3), ("L4", 4)):
        k = CONV_SIZE[l]
        lo, hi = LAYER_SL[l]
        T = (hi - lo) // k
        r = np.zeros((NIDX, T), np.float32)
        r[ONES_ROW] = 1.0
        for j in range(k):
            r[j * 4 + 0] = value[b, lo:hi][j::k]
            for s in range(3):
                r[j * 4 + 1 + s] = position[b, lo:hi, s][j::k]
        out[name] = r.astype(_BF16)
    return out


_CACHE = {}

# schedule tuning knobs (sweepable via analyze_sweep.py)
PAIR = 1  # chunks fused per eq op
BPS_BUFS = 5
OPS_BUFS = 3
MH_BUFS = 3
ACT_MOD = 4  # pair p goes to ACT when p % ACT_MOD == ACT_MOD - 1
DEPTH = 2


def _get_nc(layers, nchunks, reps=1):
    key = ("v1", PAIR, BPS_BUFS, OPS_BUFS, MH_BUFS, ACT_MOD, DEPTH, reps,
           tuple((n, t, o, tuple(c)) for n, t, o, c in layers))
    if key in _CACHE:
        return _CACHE[key]

    import concourse.bass as bass
    import concourse.tile as tile
    from concourse import bacc, mybir
    from contextlib import ExitStack

    f32 = mybir.dt.float32
    f32r = mybir.dt.float32r
    bf16 = mybir.dt.bfloat16

    nc = bacc.Bacc(trn_type="TRN2", target_bir_lowering=False, debug=False)
    tbl_d = nc.dram_tensor("tbl", [128, nchunks * E], f32r, kind="ExternalInput").ap()
    sel_d = nc.dram_tensor(
        "sel", [NIDX, nchunks * 128], bf16, kind="ExternalInput"
    ).ap()
    ridx_d = {
        name: nc.dram_tensor(f"ridx_{name}", [NIDX, T], bf16, kind="ExternalInput").ap()
        for name, T, _, _ in layers
    }
    out_d = nc.dram_tensor("out", [OUT_TOKENS, E], f32, kind="ExternalOutput").ap()

    with tile.TileContext(nc) as tc, ExitStack() as ctx:
        cpool = ctx.enter_context(tc.tile_pool(name="const", bufs=1))
        rpool = ctx.enter_context(tc.tile_pool(name="ridx", bufs=DEPTH + 1))
        mpool = ctx.enter_context(tc.tile_pool(name="mh", bufs=MH_BUFS))
        tpool = ctx.enter_context(tc.tile_pool(name="sq", bufs=3))
        bps = ctx.enter_context(
            tc.tile_pool(name="bps", bufs=BPS_BUFS, space=bass.MemorySpace.PSUM)
        )
        ops = ctx.enter_context(
            tc.tile_pool(name="ops", bufs=OPS_BUFS, space=bass.MemorySpace.PSUM)
        )
        opool = ctx.enter_context(tc.tile_pool(name="osb", bufs=3))

        # small consts first so the first broadcast matmuls start immediately;
        # the big table load is split per-layer in use order behind them
        sel_t = cpool.tile([NIDX, nchunks * 128], bf16, tag="sel")
        nc.sync.dma_start(sel_t[:], sel_d[:])
        tbl_t = cpool.tile([128, nchunks * E], f32r, tag="tbl")
        for _, _, _, cids in layers:
            lo, hi = cids[0] * E, (cids[-1] + 1) * E
            nc.sync.dma_start(tbl_t[:, lo:hi], tbl_d[:, lo:hi])

        A = mybir.ActivationFunctionType
        stripes = []
        for name, T, out_off, cids in layers:
            for s0 in range(0, T, STRIPE):
                stripes.append((name, out_off, cids, s0, min(STRIPE, T - s0)))
        # spread the small eq-heavy stripes (B/L3) between PE-heavy L4 ones
        big = [s for s in stripes if s[0] == "L4"]
        small = [s for s in stripes if s[0] != "L4"]
        small.sort(key=lambda s: -s[4])  # tiny tail stripe goes last
        stripes = []
        for i, b in enumerate(big):
            stripes.append(b)
            if i * len(small) // len(big) < (i + 1) * len(small) // len(big):
                stripes.append(small[i * len(small) // len(big)])

        def load_ridx(si):
            name, _, cids, s0, W = stripes[si]
            rt = rpool.tile([NIDX, W], bf16, tag="r")
            nc.sync.dma_start(rt[:], ridx_d[name][:, s0 : s0 + W])
            return rt

        def build_mh_pair(si, rt, p, ks):
            """broadcast matmuls + eq for a pair (or single) of chunks.

            The broadcast output is already idx - c (ones-row trick), so the
            one-hot is a compare against immediate 0 and one DVE/ACT op can
            span both chunks of the pair.
            """
            _, _, cids, _, W = stripes[si]
            n = len(ks)
            bp = bps.tile([128, n * W], f32, tag="b")
            for i, k in enumerate(ks):
                ci = cids[k]
                nc.tensor.matmul(
                    bp[:, i * W : (i + 1) * W],
                    sel_t[:, ci * 128 : (ci + 1) * 128],
                    rt[:],
                    start=True,
                    stop=True,
                )
            mh = mpool.tile([128, n * W], f32r, tag=f"mh{p}")
            if p % ACT_MOD == ACT_MOD - 1:
                # ACT path: relu(1 - x^2) — exact 0/1 for integer x
                tmp = tpool.tile([128, n * W], f32, tag="sq")
                nc.scalar.activation(tmp[:], bp[:], A.Square)
                nc.scalar.activation(mh[:], tmp[:], A.Relu, bias=1.0, scale=-1.0)
            else:
                nc.vector.tensor_scalar(
                    mh[:], bp[:], 0.0, None, op0=mybir.AluOpType.is_equal
                )
            return [mh[:, i * W : (i + 1) * W] for i in range(n)]

        def main_ttile(si, mhs, ti, ob):
            _, _, cids, _, W = stripes[si]
            t0 = ti * 128
            M = min(128, W - t0)
            op = ops.tile([M, E], f32, tag="o")
            for k, ci in enumerate(cids):
                nc.tensor.matmul(
                    op[:],
                    mhs[k][:, t0 : t0 + M],
                    tbl_t[:, ci * E : (ci + 1) * E],
                    start=(k == 0),
                    stop=(k == len(cids) - 1),
                )
            nc.scalar.activation(ob[:M, ti * E : (ti + 1) * E], op[:], A.Copy)

        def store_out(si, ob):
            _, out_off, _, s0, W = stripes[si]
            row = out_off + s0
            if W % 128 == 0:
                dst = out_d[row : row + W, :].rearrange("(a p) e -> p a e", p=128)
                src = ob[:].rearrange("p (a e) -> p a e", e=E)
                nc.sync.dma_start(dst, src)
            else:
                nc.sync.dma_start(out_d[row : row + W, :], ob[:W, :E])

        # two-stripe software pipeline with interleaved emission: pair-builds
        # of stripe s+2's one-hots alternate with stripe s's main t-tiles.
        def stripe_pairs(si):
            nk = len(stripes[si][2])
            return [tuple(range(a, min(a + PAIR, nk))) for a in range(0, nk, PAIR)]

        def emit_pairs(si, rt, prs):
            mhs = []
            for p, ks in prs:
                mhs += build_mh_pair(si, rt, p, ks)
            return mhs

        def emit_body():
            nst = len(stripes)
            mh_of = {}
            for si in range(min(DEPTH, nst)):
                rt = load_ridx(si)
                mh_of[si] = emit_pairs(si, rt, list(enumerate(stripe_pairs(si))))
            for si in range(nst):
                W = stripes[si][4]
                ntt = -(-W // 128)
                ob = opool.tile([128, ntt * E], f32, tag="ob")
                sj = si + DEPTH
                if sj < nst:
                    rt = load_ridx(sj)
                    prs = list(enumerate(stripe_pairs(sj)))
                    npr = len(prs)
                    # split stripe sj's pair-builds into ntt groups interleaved
                    # with stripe si's main t-tiles (PE fills bank-wait time)
                    bounds = [round(g * npr / ntt) for g in range(ntt + 1)]
                    mh_of[sj] = []
                    for ti in range(ntt):
                        main_ttile(si, mh_of[si], ti, ob)
                        mh_of[sj] += emit_pairs(
                            sj, rt, prs[bounds[ti] : bounds[ti + 1]]
                        )
                else:
                    for ti in range(ntt):
                        main_ttile(si, mh_of[si], ti, ob)
                store_out(si, ob)
                del mh_of[si]

        if reps == 1:
            emit_body()
        else:
            # timing mode: repeat the body on-device to measure per-iter HW
            # time as a wall-clock slope (no NTFF profiling available)
            with tc.For_i(0, reps, 1):
                emit_body()

    nc.compile()
    _CACHE[key] = nc
    return nc


def kernel(**inputs):
    from concourse.bass_utils import run_bass_kernel_spmd

    value = np.asarray(inputs["value"], np.int32).astype(np.float32)
    depth = np.asarray(inputs["depth"], np.int32).astype(np.float32)
    position = np.asarray(inputs["position"], np.int32).astype(np.float32)

    tbl, sel, layers = _build_consts(inputs)
    nc = _get_nc(layers, tbl.shape[1] // E)

    in_maps = []
    for b in range(BATCH):
        rid = _build_ridx(value, depth, position, b)
        m = {"tbl": tbl, "sel": sel}
        for name, _, _, _ in layers:
            m[f"ridx_{name}"] = rid[name]
        in_maps.append(m)

    res = run_bass_kernel_spmd(nc, in_maps, list(range(BATCH)))
    return np.stack([res.results[b]["out"] for b in range(BATCH)])
